# revision 69
# baseline (speedup 1.0000x reference)
"""Trainium2 (8 NeuronCores) kernel for a gated-attention transformer block.

Reference computation (per batch b):
    q = x@Wq, [k|v] = x@Wkv, heads=8, dh=64
    attn = softmax(q k^T / 8) v
    out  = (attn * sigmoid(x@Wg + bg)) @ Wo + bo + x
    out  = LayerNorm(out) * gamma + beta

Sharding: 8 cores = 4 batches x 2 sequence-halves. Each core computes
k/v for its full batch (duplicated across the half-pair; avoids any
collective) and q/gates/output for its own 1024 rows. Row order of
keys/values is irrelevant to attention, so each core receives x[b]
rolled so its own rows come first; compile-time indices are then
identical across cores (SPMD-safe).

On-chip layout: activations transposed ([feature, seq]) for projections
and attention; dots computed as dotsT[j, i] with 2x row-tiled bf16
matmuls (K=64 head pairs on PE quadrants). The attn@v matmuls run in
fp8e4m3 DoubleRow mode (two key-tiles contracted per instruction at
half cycle cost): the softmax exp writes attention weights directly as
fp8 into per-key-tile-pair buffers, and v is stored fp8 as
[key, jtpair, head, 2, 80] (dh padded 65->80 for the 16B dual-row
stride rule, col 64 = ones for the softmax denominator). Gating applies
head PAIRS stacked on 128 partitions (even head on 0:64, odd on
64:128), so the denominator divide and sigmoid multiply are single
full-width DVE ops and the sigmoid values are written once into their
final layout. Final Wo projection back to natural layout for the
residual + LayerNorm tail; x rows for the residual persist in SBUF from
the input pipeline instead of being re-fetched from HBM.

Scheduling: projections for head-pair p+1 are emitted interleaved with
attention of pair p so the TensorEngine stays busy while the ScalarEngine
runs the (bottleneck) softmax exponentials. All sigmoids are emitted
before the first exp and the LayerNorm sqrts after the last one, so the
ScalarEngine's activation table is switched exactly twice.
"""

import sys
import os
import numpy as np

for _p in ("/opt/trn_rl_repo", "/root/.axon_site/_ro/trn_rl_repo"):
    if os.path.isdir(_p) and _p not in sys.path:
        sys.path.insert(0, _p)

import concourse.bass as bass
import concourse.tile as tile
from concourse import bacc, mybir
from concourse.bass_utils import run_bass_kernel_spmd
from concourse.masks import make_identity

F32 = mybir.dt.float32
F32R = mybir.dt.float32r
BF16 = mybir.dt.bfloat16
FP8 = mybir.dt.float8e4
AF = mybir.ActivationFunctionType
OP = mybir.AluOpType
DR = mybir.MatmulPerfMode.DoubleRow

B, N, D, H, DH = 4, 2048, 512, 8, 64
NH = N // 2          # rows owned per core
NJT = N // 128       # 16 key tiles
NJP = NJT // 2       # 8 key tile pairs
DHP = 80             # dh + denom col + pad to 16B dual-row stride
SCALE = DH ** -0.5   # 0.125
EPS = 1e-5
NCORES = 8


def build_nc(trivial_bo=False, trivial_gb=False):
    nc = bacc.Bacc("TRN2", target_bir_lowering=False, debug=False,
                   num_devices=NCORES)

    xkv = nc.dram_tensor("xkv", [N, D], F32, kind="ExternalInput")
    Wq = nc.dram_tensor("Wq", [D, D], F32, kind="ExternalInput")
    Wk = nc.dram_tensor("Wk", [D, D], F32, kind="ExternalInput")
    Wv = nc.dram_tensor("Wv", [D, D], F32, kind="ExternalInput")
    Wg = nc.dram_tensor("Wg", [D, D], F32, kind="ExternalInput")
    Wo = nc.dram_tensor("Wo", [D, D], F32, kind="ExternalInput")
    bg = nc.dram_tensor("bg", [D], F32, kind="ExternalInput")
    bo = nc.dram_tensor("bo", [D], F32, kind="ExternalInput")
    gamma = nc.dram_tensor("gamma", [D], F32, kind="ExternalInput")
    beta = nc.dram_tensor("beta", [D], F32, kind="ExternalInput")
    out = nc.dram_tensor("out", [NH, D], F32, kind="ExternalOutput")

    def bcast_ap(t, n):
        return bass.AP(tensor=t, offset=0, ap=[[0, 128], [1, n]])

    with tile.TileContext(nc) as tc:
        with tc.tile_pool(name="consts", bufs=1) as consts, \
             tc.tile_pool(name="wpool", bufs=1) as wpool, \
             tc.tile_pool(name="acts", bufs=1) as acts, \
             tc.tile_pool(name="stage", bufs=2) as stage, \
             tc.tile_pool(name="prpool", bufs=4) as prpool, \
             tc.tile_pool(name="ppool", bufs=2, space="PSUM") as ppool, \
             tc.tile_pool(name="papool", bufs=2, space="PSUM") as papool, \
             tc.tile_pool(name="pmisc", bufs=2, space="PSUM") as pmisc:

            # ---- constants ----
            ident = consts.tile([128, 128], F32)
            make_identity(nc, ident[:])
            eps_t = consts.tile([128, 1], F32)
            nc.vector.memset(eps_t[:], EPS)

            # ---- weights: loaded fp32 and used directly as float32r matmul
            #      operands (full rate at moving-dim >= 256) — no casts at
            #      all. Chunked DMAs for Wk (fastest time-to-ready); one
            #      large strided DMA for the rest. Wq rides the scalar queue
            #      so Wk and Wq stream in parallel. ----
            w_fp = {}

            def load_weight(name, t, eng=None, cast_eng=None, split_m0=False):
                def emit():
                    ce = cast_eng or nc.vector
                    wb = wpool.tile([128, 4, D], BF16, tag=f"w_{name}")
                    ws = stage.tile([128, 4, D], F32, tag="wstage", bufs=2)
                    (eng or nc.sync).dma_start(
                        ws[:], t.ap().rearrange("(c p) d -> p c d", p=128))
                    if split_m0:
                        # cast only the first output-column block now: it is
                        # all pair-0's projections read, and it unblocks the
                        # first dots ~2us earlier than a full-matrix cast
                        ce.tensor_copy(wb[:, :, 0:128], ws[:, :, 0:128])
                        w_rest[name] = (wb, ws)
                    else:
                        ce.tensor_copy(wb[:], ws[:])
                    w_fp[name] = wb
                return emit

            w_rest = {}

            def cast_rest(name, cast_eng=None):
                ce = cast_eng or nc.vector
                wb, ws = w_rest.pop(name)
                ce.tensor_copy(wb[:, :, 128:D], ws[:, :, 128:D])

            nbg = consts.tile([128, 4], F32)
            nc.sync.dma_start(nbg[:], bg.ap().rearrange("(m p) -> p m", p=128))
            nc.vector.tensor_scalar_mul(nbg[:], nbg[:], -1.0)

            # ---- tensors for x / projections ----
            xT = acts.tile([128, 4, N], BF16)
            xresid = acts.tile([128, 8, D], F32)   # own rows kept for residual
            sig128 = acts.tile([128, 4, NH], BF16)  # head pair stacked 64+64
            qT = acts.tile([128, 4, NH], BF16)
            kT = acts.tile([128, 4, N], BF16)
            # v in fp8, two key-tiles (DoubleRow groups) per jp slot.
            # Only the pad+ones columns need initialization (v_unit writes
            # 0:64); DVE memsets run before the weight casts enter its queue.
            v3 = acts.tile([128, NJP, H, 2, DHP], FP8)
            for jp in range(NJP):
                nc.gpsimd.memset(v3[:, jp, :, :, DH:DHP], 0.0)
            for jp in range(NJP):
                nc.gpsimd.memset(v3[:, jp, :, :, DH:DH + 1], 1.0)

            def gates_unit(m, ic):
                # sigmoid(g+bg) = 1/(1+exp(-(g+bg))) -- uses the Exp table so
                # these can interleave freely with the attention exps
                def emit():
                    pm = pmisc.tile([128, 512], F32, tag="m")
                    for kc in range(4):
                        nc.tensor.matmul(pm[:], w_fp["Wg"][:, kc, m * 128:(m + 1) * 128],
                                         xT[:, kc, ic * 512:(ic + 1) * 512],
                                         start=(kc == 0), stop=(kc == 3))
                    e = stage.tile([128, 512], F32, tag="gexp")
                    nc.scalar.activation(e[:], pm[:], AF.Exp, scale=-1.0,
                                         bias=nbg[:, m:m + 1])
                    nc.vector.tensor_scalar_add(e[:], e[:], 1.0)
                    with nc.allow_low_precision(reason="sigmoid in [0.2,1): bf16 ok"):
                        nc.vector.reciprocal(
                            sig128[:, m, ic * 512:(ic + 1) * 512], e[:])
                return emit

            def qt_unit(m, ic):
                def emit():
                    pm = pmisc.tile([128, 512], F32, tag="m")
                    for kc in range(4):
                        nc.tensor.matmul(pm[:], w_fp["Wq"][:, kc, m * 128:(m + 1) * 128],
                                         xT[:, kc, ic * 512:(ic + 1) * 512],
                                         start=(kc == 0), stop=(kc == 3))
                    nc.vector.tensor_copy(qT[:, m, ic * 512:(ic + 1) * 512], pm[:])
                return emit

            def kt_unit(m, ic):
                def emit():
                    pm = pmisc.tile([128, 512], F32, tag="m")
                    for kc in range(4):
                        nc.tensor.matmul(pm[:], w_fp["Wk"][:, kc, m * 128:(m + 1) * 128],
                                         xT[:, kc, ic * 512:(ic + 1) * 512],
                                         start=(kc == 0), stop=(kc == 3))
                    nc.vector.tensor_copy(kT[:, m, ic * 512:(ic + 1) * 512], pm[:])
                return emit

            def v_unit(jt, evac_eng=None):
                def emit():
                    pm = pmisc.tile([128, 512], F32, tag="m")
                    for kc in range(4):
                        nc.tensor.matmul(pm[:], xT[:, kc, jt * 128:(jt + 1) * 128],
                                         w_fp["Wv"][:, kc, :],
                                         start=(kc == 0), stop=(kc == 3))
                    ee = evac_eng or nc.vector
                    if ee is nc.scalar:
                        nc.scalar.copy(
                            v3[:, jt // 2, :, jt % 2, 0:DH],
                            pm[:].rearrange("p (h d) -> p h d", h=H))
                    else:
                        ee.tensor_copy(
                            v3[:, jt // 2, :, jt % 2, 0:DH],
                            pm[:].rearrange("p (h d) -> p h d", h=H))
                return emit

            def dots_step(p, ic, jt):
                pd = ppool.tile([128, 1024], F32)
                nc.tensor.matmul(pd[:, 0:512],
                                 kT[0:64, p, jt * 128:(jt + 1) * 128],
                                 qT[0:64, p, ic * 512:(ic + 1) * 512],
                                 start=True, stop=True,
                                 tile_position=(0, 0))
                nc.tensor.matmul(pd[:, 512:1024],
                                 kT[64:128, p, jt * 128:(jt + 1) * 128],
                                 qT[64:128, p, ic * 512:(ic + 1) * 512],
                                 start=True, stop=True,
                                 tile_position=(64, 0))
                return pd

            # ---- x: four grouped DMAs of 4 row-tiles each, all issued on
            #      the gpsimd queue before anything can block it; transposes
            #      (fp32 on the PE) start after the first group. Own rows
            #      (groups A/B -> xresid) persist in SBUF for the residual
            #      tail. The scalar queue carries only Wq before the exps;
            #      sync carries Wk/Wv/Wg/Wo. ----
            xgC = stage.tile([128, 4, D], F32, tag="xgC", bufs=1)
            xgD = stage.tile([128, 4, D], F32, tag="xgD", bufs=1)
            nc.sync.dma_start(
                xresid[:, 0:4, :], xkv[0:512, :].rearrange("(c p) d -> p c d", p=128))

            def x_transpose(nt):
                def emit():
                    if nt < 8:
                        xs = xresid[:, nt, :]
                    elif nt < 12:
                        xs = xgC[:, nt - 8, :]
                    else:
                        xs = xgD[:, nt - 12, :]
                    pt = pmisc.tile([128, 4, 128], F32, tag="m")
                    for kc in range(4):
                        nc.tensor.transpose(pt[:, kc, :],
                                            xs[:, kc * 128:(kc + 1) * 128],
                                            ident[:])
                    # one evacuation for all four transposes: on the
                    # ScalarEngine during the prologue (it idles there; Copy
                    # is in every ACT table set), on the DVE mid-attention
                    dst = xT[:, :, nt * 128:(nt + 1) * 128]
                    if nt < 4:
                        nc.scalar.copy(dst, pt[:])
                    else:
                        nc.vector.tensor_copy(dst, pt[:])
                return emit

            load_weight("Wk", Wk, split_m0=True)()
            load_weight("Wq", Wq, split_m0=True)()
            nc.sync.dma_start(
                xresid[:, 4:8, :], xkv[512:1024, :].rearrange("(c p) d -> p c d", p=128))
            load_weight("Wv", Wv, cast_eng=nc.gpsimd)()
            nc.sync.dma_start(
                xgC[:], xkv[1024:1536, :].rearrange("(c p) d -> p c d", p=128))
            nc.sync.dma_start(
                xgD[:], xkv[1536:2048, :].rearrange("(c p) d -> p c d", p=128))
            load_weight("Wg", Wg, cast_eng=nc.gpsimd)()
            for nt in range(4):
                x_transpose(nt)()
            kt_unit(0, 0)()
            qt_unit(0, 0)()
            pd_q = [dots_step(0, 0, 0), dots_step(0, 0, 1)]
            for nt in range(4, 8):
                x_transpose(nt)()
            kt_unit(0, 1)()
            qt_unit(0, 1)()
            v_unit(0)()
            v_unit(1)()
            cast_rest("Wk")
            cast_rest("Wq")
            # Wo stacked like gatedT: head 2m rows on partitions 0:64, head
            # 2m+1 on 64:128, so each head pair contracts in ONE K=128 matmul
            wo_s = stage.tile([128, 4, D], F32, tag="wostage", bufs=1)
            for h in range(H):
                lo = (h % 2) * 64
                nc.sync.dma_start(wo_s[lo:lo + 64, h // 2, :],
                                  Wo[h * 64:(h + 1) * 64, :])
            wo_b = wpool.tile([128, 4, D], BF16)
            nc.gpsimd.tensor_copy(wo_b[:], wo_s[:])

            # during pair p's attention, emit projections for pair p+1
            # (v3 for the remaining jt is finished inside pair-0 ic=0,
            # pipelined two key-tiles ahead of its consumer)
            queues = {
                0: [kt_unit(1, ic) for ic in range(4)]
                   + [qt_unit(1, ic) for ic in range(2)]
                   + [gates_unit(1, 0), gates_unit(1, 1)],
                1: [kt_unit(2, ic) for ic in range(4)]
                   + [qt_unit(2, ic) for ic in range(2)]
                   + [gates_unit(2, 0), gates_unit(2, 1)],
                2: [kt_unit(3, ic) for ic in range(4)]
                   + [qt_unit(3, ic) for ic in range(2)]
                   + [gates_unit(3, 0), gates_unit(3, 1)],
                3: None,  # filled per-ic below: Wo/LN for it 0..3 during ic=1
            }

            # ---- attention, per head pair ----
            # gated attn output, head pairs stacked: even head on partitions
            # 0:64 (from pe_), odd head on 64:128 (from po_)
            gatedT = acts.tile([128, 4, NH], BF16)

            # wo units are split: the "front" (Wo matmuls, residual add, LN
            # statistics on the DVE) can interleave with attention without
            # touching the ScalarEngine (no act table switch mid-exp-stream);
            # the "tail" (rsqrt, normalize, store) runs after the last exp.
            wo_state = {}

            def wo_front(it, psum="m", hold=False, act_ln=False):
                def emit():
                    if psum == "att":
                        pw = papool.tile([128, 512], F32, tag="att")
                    elif psum == "pd":
                        pw_full = ppool.tile([128, 1024], F32, tag="pd")
                        pw = pw_full[:, 0:512]
                    else:
                        pw = pmisc.tile([128, 512], F32, tag="m")
                    mhi = 3 if not hold else 2
                    for m in range(mhi + 1):
                        nc.tensor.matmul(
                            pw[:],
                            gatedT[:, m, it * 128:(it + 1) * 128],
                            wo_b[:, m, :], start=(m == 0), stop=(m == mhi))
                    if hold:
                        wo_state[it] = {"pw": pw}
                        return
                    _wo_stats(it, pw, act_ln)
                return emit

            def _wo_stats(it, pw, act_ln):
                y = stage.tile([128, D], F32, tag=f"y{it}", bufs=1)
                nc.vector.tensor_add(y[:], pw[:], xresid[:, it, :])
                if not trivial_bo:
                    nc.vector.tensor_add(y[:], y[:], bo_b[:])
                ve = stage.tile([128, 1], F32, tag=f"ve{it}", bufs=1)
                if act_ln:
                    # LN statistics on the (tail-idle) ScalarEngine:
                    # accum_out gives per-row sum / sum-of-squares
                    cp = stage.tile([128, D], F32, tag="lncp", bufs=1)
                    sm = stage.tile([128, 2], F32, tag=f"sm{it}", bufs=1)
                    nc.scalar.activation(cp[:], y[:], AF.Copy,
                                         accum_out=sm[:, 0:1])
                    nc.scalar.activation(cp[:], y[:], AF.Square,
                                         accum_out=sm[:, 1:2])
                    mu = stage.tile([128, 1], F32, tag=f"mu{it}", bufs=1)
                    nc.vector.tensor_scalar_mul(mu[:], sm[:, 0:1], 1.0 / D)
                    m2 = stage.tile([128, 1], F32, tag="m2T")
                    nc.vector.tensor_mul(m2[:], mu[:], mu[:])
                    nc.vector.tensor_scalar_mul(ve[:], sm[:, 1:2], 1.0 / D)
                    nc.vector.tensor_sub(ve[:], ve[:], m2[:])
                    nc.vector.tensor_add(ve[:], ve[:], eps_t[:])
                    mu_ap = mu[:]
                else:
                    st = stage.tile([128, 6], F32, tag="st")
                    nc.vector.bn_stats(st[:], y[:])
                    mv = stage.tile([128, 2], F32, tag=f"mv{it}", bufs=1)
                    nc.vector.bn_aggr(mv[:], st[:])
                    nc.vector.tensor_add(ve[:], mv[:, 1:2], eps_t[:])
                    mu_ap = mv[:, 0:1]
                nc.vector.reciprocal(ve[:], ve[:])
                wo_state[it] = {"y": y, "ve": ve, "mu": mu_ap}

            def wo_tail(it, store_eng=None, z_eng=None):
                def emit():
                    s = wo_state[it]
                    y, ve, mu_ap = s["y"], s["ve"], s["mu"]
                    nc.scalar.activation(ve[:], ve[:], AF.Sqrt)
                    ze = z_eng or nc.vector
                    ze.tensor_scalar(y[:], y[:], mu_ap, ve[:],
                                     OP.subtract, OP.mult)
                    if not trivial_gb:
                        ze.tensor_mul(y[:], y[:], gam_b[:])
                        ze.tensor_add(y[:], y[:], bet_b[:])
                    eng = store_eng or nc.sync
                    eng.dma_start(out[it * 128:(it + 1) * 128, :], y[:])
                return emit

            def wo_finish(it, act_ln=True):
                def emit():
                    pw = wo_state[it]["pw"]
                    nc.tensor.matmul(pw[:], gatedT[:, 3, it * 128:(it + 1) * 128],
                                     wo_b[:, 3, :], start=False, stop=True,
                                     skip_group_check=True)
                    _wo_stats(it, pw, act_ln)
                return emit

            def gating(p, ic, tail=False):
                # reciprocals first so the Pool broadcasts overlap the raw
                # copies; head pair stacked onto 128 partitions, then one
                # full-width divide + gate multiply. The last block reads
                # PSUM directly (latency over PSUM-freeing, which no longer
                # matters there).
                pe_, po_ = acc_tiles[(p, ic)]
                r0e = stage.tile([1, 512], F32, tag="r0e")
                r0o = stage.tile([1, 512], F32, tag="r0o")
                nc.vector.reciprocal(r0e[:], pe_[64:65, :])
                nc.vector.reciprocal(r0o[:], po_[64:65, :])
                # partition_broadcast only writes correctly at partition
                # offset 0: fill two full-height tiles and keep every DVE op
                # partition-aligned across its operands
                rb_e = stage.tile([128, 512], F32, tag="rb")
                rb_o = stage.tile([128, 512], F32, tag="rbo")
                nc.gpsimd.partition_broadcast(rb_e[:], r0e[:])
                nc.gpsimd.partition_broadcast(rb_o[:], r0o[:])
                raw = stage.tile([128, 512], F32, tag="praw", bufs=1)
                if tail:
                    # the ScalarEngine idles after its table switch: give it
                    # the unshifted half of the evacuation
                    nc.scalar.copy(raw[0:64, :], pe_[0:64, :])
                else:
                    nc.vector.tensor_copy(raw[0:64, :], pe_[0:64, :])
                nc.vector.tensor_copy(raw[64:128, :], po_[0:64, :])
                tmp = stage.tile([128, 512], F32, tag="tmp")
                nc.vector.tensor_mul(tmp[0:64, :], raw[0:64, :], rb_e[0:64, :])
                nc.vector.tensor_mul(tmp[64:128, :], raw[64:128, :], rb_o[64:128, :])
                nc.vector.tensor_mul(gatedT[:, p, ic * 512:(ic + 1) * 512],
                                     tmp[:], sig128[:, p, ic * 512:(ic + 1) * 512])

            # flat block schedule with cross-block dots prefetch: the first
            # two dots of block b+1 issue during block b's last two exps so
            # the exp stream never waits on dots at a block boundary
            blocks = [(p, ic) for p in range(4) for ic in range(2)]
            acc_tiles = {}
            wis = {}
            for bi, (p, ic) in enumerate(blocks):
                nxt = blocks[bi + 1] if bi + 1 < len(blocks) else None
                if (p, ic) == (3, 1):
                    work = [wo_front(it, act_ln=False) for it in range(4)] \
                         + [wo_front(4, hold=True), wo_front(5, hold=True)]
                    wi = 0
                else:
                    work = wis.setdefault(p, {"q": queues[p] or [], "i": 0})
                pe_ = papool.tile([128, 512], F32, tag="att")
                po_ = papool.tile([128, 512], F32, tag="att")
                acc_tiles[(p, ic)] = (pe_, po_)
                pr_t = None
                for jt in range(NJT):
                    if jt % 2 == 0:
                        pr_t = prpool.tile([128, 2, 2, 512], FP8, tag="pr")
                    nc.scalar.activation(
                        pr_t[:, :, jt % 2, :],
                        pd_q.pop(0)[:].rearrange("p (h x) -> p h x", h=2),
                        AF.Exp, scale=SCALE)
                    if jt + 2 < NJT:
                        pd_q.append(dots_step(p, ic, jt + 2))
                    elif nxt is not None:
                        pd_q.append(dots_step(nxt[0], nxt[1], jt + 2 - NJT))
                    if p == 0 and ic == 0:
                        if jt < 8:
                            x_transpose(8 + jt)()
                        if jt == 4:
                            kt_unit(0, 2)()
                        elif jt == 8:
                            kt_unit(0, 3)()
                        if jt + 2 < NJT:
                            v_unit(jt + 2)()
                        if jt == 14:
                            gates_unit(0, 0)()
                            gates_unit(0, 1)()
                    elif (p, ic) == (3, 1):
                        if wi < len(work) and jt % 2 == 0 and jt <= 10:
                            work[wi]()
                            wi += 1
                        elif jt == 14:
                            # pd slots are free for good here: pre-accumulate
                            # wo6's first three head pairs on one of them
                            wo_front(6, psum="pd", hold=True)()

                    else:
                        if work["i"] < len(work["q"]) and jt % 2 == 0:
                            work["q"][work["i"]]()
                            work["i"] += 1
                    if jt % 2 == 1:
                        jp = jt // 2
                        nc.tensor.matmul(pe_[0:DHP, :], v3[:, jp, 2 * p, :, :],
                                         pr_t[:, 0, :, :],
                                         start=(jp == 0), stop=(jp == NJP - 1),
                                         perf_mode=DR)
                        nc.tensor.matmul(po_[0:DHP, :], v3[:, jp, 2 * p + 1, :, :],
                                         pr_t[:, 1, :, :],
                                         start=(jp == 0), stop=(jp == NJP - 1),
                                         perf_mode=DR)
                if (p, ic) != (3, 1):
                    gating(p, ic)

            # ---- tail: final gating, then the 8 LayerNorm+store units.
            #      One act-table switch (Exp->Sqrt; Copy/Square live in the
            #      sqrt set too, so the act_ln stats don't switch back).
            #      z-normalize on DVE for it 0..3 and on Pool for 4..7;
            #      stores spread over sync/scalar/gpsimd queues. ----
            # it 0..3 are fully determined before the final gating: their
            # normalize+store go first (z on DVE before the gating chain's
            # DVE work for 0/1, on Pool after the broadcasts for 2/3) so
            # those stores stream out while pair-3's gating still runs.
            gating(3, 1, tail=True)
            wo_tail(0, store_eng=nc.sync, z_eng=nc.gpsimd)()
            wo_tail(1, store_eng=nc.gpsimd, z_eng=nc.gpsimd)()
            wo_tail(2, store_eng=nc.sync, z_eng=nc.gpsimd)()
            wo_tail(3, store_eng=nc.gpsimd, z_eng=nc.gpsimd)()
            wo_finish(4, act_ln=True)()
            wo_finish(5, act_ln=True)()
            wo_finish(6, act_ln=True)()
            wo_front(7, psum="att", act_ln=True)()
            wo_tail(4, store_eng=nc.sync)()
            wo_tail(5, store_eng=nc.gpsimd)()
            wo_tail(6, store_eng=nc.sync)()
            wo_tail(7, store_eng=nc.scalar, z_eng=nc.gpsimd)()


    nc.compile()
    return nc


_NC_CACHE = {}


def _get_nc(trivial_bo=False, trivial_gb=False):
    key = (trivial_bo, trivial_gb)
    if key not in _NC_CACHE:
        _NC_CACHE[key] = build_nc(*key)
    return _NC_CACHE[key]


def kernel(**inputs) -> np.ndarray:
    x = np.asarray(inputs["x"], dtype=np.float32)
    Wq = np.ascontiguousarray(np.asarray(inputs["Wq"], dtype=np.float32))
    Wkv = np.asarray(inputs["Wkv"], dtype=np.float32)
    Wk = np.ascontiguousarray(Wkv[:, :D])
    Wv = np.ascontiguousarray(Wkv[:, D:])
    Wg = np.ascontiguousarray(np.asarray(inputs["Wg"], dtype=np.float32))
    Wo = np.ascontiguousarray(np.asarray(inputs["Wo"], dtype=np.float32))
    bg = np.ascontiguousarray(np.asarray(inputs["bg"], dtype=np.float32))
    bo = np.ascontiguousarray(np.asarray(inputs["bo"], dtype=np.float32))
    gamma = np.ascontiguousarray(np.asarray(inputs["gamma"], dtype=np.float32))
    beta = np.ascontiguousarray(np.asarray(inputs["beta"], dtype=np.float32))

    trivial_bo = bool(np.all(bo == 0.0))
    trivial_gb = bool(np.all(gamma == 1.0) and np.all(beta == 0.0))
    nc = _get_nc(trivial_bo, trivial_gb)
    in_maps = []
    for c in range(NCORES):
        b, half = c // 2, c % 2
        rolled = np.ascontiguousarray(np.roll(x[b], -half * NH, axis=0))
        in_maps.append({"xkv": rolled, "Wq": Wq, "Wk": Wk, "Wv": Wv,
                        "Wg": Wg, "Wo": Wo, "bg": bg, "bo": bo,
                        "gamma": gamma, "beta": beta})
    res = run_bass_kernel_spmd(nc, in_maps, core_ids=list(range(NCORES)))
    out = np.empty((B, N, D), dtype=np.float32)
    for c in range(NCORES):
        b, half = c // 2, c % 2
        out[b, half * NH:(half + 1) * NH] = res.results[c]["out"]
    return out
